# revision 3
# baseline (speedup 1.0000x reference)
"""Trainium2 Bass kernel v2 for nn_Attention_52982716563627.

Reference math (quirks preserved): qkv = x@W_atten + b; per-head scores
s = q k^T (no 1/sqrt(dk)); mask as w*m + (1-m)*1e5; softmax over the HEAD
axis; a = p@v; out = a@W_proj + b_proj.

Softmax-over-heads identities used:
  * masked (k,i): all 16 heads equal -> p = 1/16 exactly.
  * e = exp(s) computed raw; T = sum_h e; D = T + mz where mz = 16*(1-m)
    (masked entries: D irrelevant-but-finite); rbm = m/D; p_h = e_h*rbm
    (0 on masked entries).
  * the missing masked mass (1-m)/16 enters via a per-chunk correction
    matmul  av += v_chunk^T @ m1s  with m1s = (1-m)/16  (exact), and for
    slot-A chunks c >= 8 via a rank-1 suffix term (1/16 * colsum v).

Uniform SPMD program (same NEFF on 8 cores; per-core behavior via data):
  core c: batch b = c//4, rank g = c%4.
  q tile A = rows [256g, 256g+256)        -> 8 key chunks + suffix(c>=8)
  q tile B = rows [256(g+4), 256(g+4)+256) -> 16 key chunks
  K/V: core computes keys [512g, 512g+512), AllGather across the 4-core
  batch group (overlaps q projection).  All matmul operands bf16
  (PE 1 cyc/row + FWL); accumulation fp32 in PSUM; e/p bf16 (2x DVE).
"""

import numpy as np
import ml_dtypes

import concourse.bass as bass
import concourse.bacc as bacc
import concourse.mybir as mybir
import concourse.tile as tile
from concourse import bass_utils

N_CORES = 8
B, S, E = 2, 2048, 1024
H, HD = 16, 64
QT = 256
KC = 128
NKC = 16
NA, NB = 8, 16          # chunks processed for tile A / tile B
FP = mybir.dt.float32
BF = mybir.dt.bfloat16
AF = mybir.ActivationFunctionType
NPBF = ml_dtypes.bfloat16

CFG = dict(
    use_ag=True,
    bcast_norm=True,
    no_corr=False,   # debug: skip m1 correction matmuls
    no_suffix=False, # debug: skip suffix rank-1 adds
    cut=0,  # 1: phase A only; 2: +kt/v load+suffix only
)


def _bcast_mid(ap, n):
    """View a [128, Q] AP as [128, n, Q] with 0-stride middle dim."""
    return bass.AP(tensor=ap.tensor, offset=ap.offset,
                   ap=[ap.ap[0], [0, n], ap.ap[1]])


def build_program(reps: int = 1, cfg=None):
    cfg = dict(CFG, **(cfg or {}))
    use_ag = cfg["use_ag"]
    nc = bacc.Bacc("TRN2", target_bir_lowering=False, debug=False,
                   num_devices=N_CORES)

    xTq_d = nc.dram_tensor("xTq", [E, 512], BF, kind="ExternalInput")
    xTk_d = nc.dram_tensor("xTk", [E, 512], BF, kind="ExternalInput")
    if not use_ag:
        xT_d = nc.dram_tensor("xT", [E, S], BF, kind="ExternalInput")
    wq_d = nc.dram_tensor("wq", [E, E], BF, kind="ExternalInput")
    wk_d = nc.dram_tensor("wk", [E, E], BF, kind="ExternalInput")
    wv_d = nc.dram_tensor("wv", [E, E], BF, kind="ExternalInput")
    wo_d = nc.dram_tensor("wo", [E, E], BF, kind="ExternalInput")
    bq_d = nc.dram_tensor("bq", [128, 8], FP, kind="ExternalInput")
    bk_d = nc.dram_tensor("bk", [128, 8], FP, kind="ExternalInput")
    bv_d = nc.dram_tensor("bv", [1, E], BF, kind="ExternalInput")
    bo_d = nc.dram_tensor("bo", [1, E], BF, kind="ExternalInput")
    ones_d = nc.dram_tensor("onesr", [1, 256], BF, kind="ExternalInput")
    zeros_d = nc.dram_tensor("zerosr", [1, 512], BF, kind="ExternalInput")
    o16_d = nc.dram_tensor("o16", [128, 1], BF, kind="ExternalInput")
    m_d = nc.dram_tensor("mm", [24, KC, QT], BF, kind="ExternalInput")
    mz_d = nc.dram_tensor("mz", [24, KC, QT], BF, kind="ExternalInput")
    m1s_d = nc.dram_tensor("m1s", [24, KC, QT], BF, kind="ExternalInput")
    out_d = nc.dram_tensor("out", [512, E], FP, kind="ExternalOutput")

    with tile.TileContext(nc) as tc:
        with (
            tc.tile_pool(name="consts", bufs=1) as consts,
            tc.tile_pool(name="qt", bufs=1) as qtp,
            tc.tile_pool(name="vsb", bufs=1) as vsbp,
            tc.tile_pool(name="ktp", bufs=1) as ktpool,
            tc.tile_pool(name="dram", bufs=1, space="DRAM") as dramp,
        ):
            ones_sb = consts.tile([1, 256], BF)
            zeros_sb = consts.tile([1, 512], BF)
            o16_sb = consts.tile([128, 1], BF)
            bq_sb = consts.tile([128, 8], FP)
            bk_sb = consts.tile([128, 8], FP)
            bv_sb = consts.tile([1, E], BF)
            bo_sb = consts.tile([1, E], BF)
            nc.sync.dma_start(ones_sb[:], ones_d[:])
            nc.sync.dma_start(zeros_sb[:], zeros_d[:])
            nc.sync.dma_start(o16_sb[:], o16_d[:])
            nc.sync.dma_start(bq_sb[:], bq_d[:])
            nc.sync.dma_start(bk_sb[:], bk_d[:])
            nc.sync.dma_start(bv_sb[:], bv_d[:])
            nc.sync.dma_start(bo_sb[:], bo_d[:])

            qt = qtp.tile([128, 8, 512], BF)
            v = vsbp.tile([128, NKC, E], BF)
            kt = ktpool.tile([128, 8, S], BF)

            if use_ag:
                kin_dr = dramp.tile([E, 512], BF)
                kout_dr = dramp.tile([4 * E, 512], BF)
                vin_dr = dramp.tile([512, E], BF)
                vout_dr = dramp.tile([S, E], BF)

            loop = tc.For_i(0, reps, 1) if reps > 1 else None
            if loop is not None:
                loop.__enter__()

            # ============ Phase A: own K/V slice -> AG; q proj ============
            with (
                tc.tile_pool(name="xts", bufs=1) as xtsp,
                tc.tile_pool(name="wstr", bufs=2) as wstr,
                tc.tile_pool(name="kvo", bufs=3) as kvop,
                tc.tile_pool(name="pskq", bufs=2, space="PSUM") as pskq,
                tc.tile_pool(name="psv", bufs=2, space="PSUM") as psv,
            ):
                if use_ag:
                    xtk = xtsp.tile([128, 8, 512], BF, name="xtk")
                    for e in range(8):
                        nc.sync.dma_start(
                            xtk[:, e, :], xTk_d[e * 128:(e + 1) * 128, :])
                else:
                    xt = xtsp.tile([128, 8, S], BF, name="xt")
                    for e in range(8):
                        nc.sync.dma_start(
                            xt[:, e, :], xT_d[e * 128:(e + 1) * 128, :])
                xtq = xtsp.tile([128, 8, 512], BF, name="xtq")
                for e in range(8):
                    nc.sync.dma_start(
                        xtq[:, e, :], xTq_d[e * 128:(e + 1) * 128, :])

                # own kT slice (AG) or full kT (fallback)
                for ct in range(8):
                    wct = wstr.tile([128, 8, 128], BF, tag="wct")
                    nc.sync.dma_start(
                        wct[:],
                        wk_d[:, ct * 128:(ct + 1) * 128].rearrange(
                            "(e p) c -> p e c", p=128))
                    if use_ag:
                        ps = pskq.tile([128, 512], FP, tag="pkq")
                        for e in range(8):
                            nc.tensor.matmul(ps[:], wct[:, e, :],
                                             xtk[:, e, :],
                                             start=(e == 0), stop=(e == 7))
                        ktp = kvop.tile([128, 512], BF, tag="ktp")
                        nc.scalar.activation(ktp[:], ps[:], AF.Identity,
                                             bias=bk_sb[:, ct:ct + 1])
                        nc.sync.dma_start(
                            kin_dr[ct * 128:(ct + 1) * 128, :], ktp[:])
                    else:
                        for kc4 in range(4):
                            ps = pskq.tile([128, 512], FP, tag="pkq")
                            for e in range(8):
                                nc.tensor.matmul(
                                    ps[:], wct[:, e, :],
                                    xt[:, e, kc4 * 512:(kc4 + 1) * 512],
                                    start=(e == 0), stop=(e == 7))
                            nc.scalar.activation(
                                kt[:, ct, kc4 * 512:(kc4 + 1) * 512], ps[:],
                                AF.Identity, bias=bk_sb[:, ct:ct + 1])
                if use_ag:
                    nc.gpsimd.collective_compute(
                        "AllGather", mybir.AluOpType.bypass,
                        replica_groups=[[0, 1, 2, 3], [4, 5, 6, 7]],
                        ins=[kin_dr.opt()], outs=[kout_dr.opt()])

                # own v slice (AG) or full v (fallback)
                for cc in range(2):
                    wvcc = wstr.tile([128, 8, 512], BF, tag="wvcc")
                    nc.sync.dma_start(
                        wvcc[:],
                        wv_d[:, cc * 512:(cc + 1) * 512].rearrange(
                            "(e p) c -> p e c", p=128))
                    nrt = 4 if use_ag else 16
                    for rt in range(nrt):
                        ps = psv.tile([128, 512], FP, tag="pv")
                        for e in range(8):
                            src = (xtk[:, e, rt * 128:(rt + 1) * 128]
                                   if use_ag else
                                   xt[:, e, rt * 128:(rt + 1) * 128])
                            nc.tensor.matmul(ps[:], src, wvcc[:, e, :],
                                             start=(e == 0), stop=False)
                        nc.tensor.matmul(ps[:], ones_sb[:1, :128],
                                         bv_sb[:1, cc * 512:(cc + 1) * 512],
                                         start=False, stop=True)
                        vo = kvop.tile([128, 512], BF, tag="vo")
                        nc.scalar.activation(vo[:], ps[:], AF.Copy)
                        if use_ag:
                            nc.sync.dma_start(
                                vin_dr[rt * 128:(rt + 1) * 128,
                                       cc * 512:(cc + 1) * 512], vo[:])
                        else:
                            nc.vector.tensor_copy(
                                v[:, rt, cc * 512:(cc + 1) * 512], vo[:])
                if use_ag:
                    nc.gpsimd.collective_compute(
                        "AllGather", mybir.AluOpType.bypass,
                        replica_groups=[[0, 1, 2, 3], [4, 5, 6, 7]],
                        ins=[vin_dr.opt()], outs=[vout_dr.opt()])

                # q projection for the two own 256-row tiles
                for ct in range(8):
                    wct = wstr.tile([128, 8, 128], BF, tag="wct")
                    nc.sync.dma_start(
                        wct[:],
                        wq_d[:, ct * 128:(ct + 1) * 128].rearrange(
                            "(e p) c -> p e c", p=128))
                    for t in range(2):
                        ps = pskq.tile([128, 256], FP, tag="pq")
                        for e in range(8):
                            nc.tensor.matmul(
                                ps[:], wct[:, e, :],
                                xtq[:, e, t * 256:(t + 1) * 256],
                                start=(e == 0), stop=(e == 7))
                        nc.scalar.activation(
                            qt[:, ct, t * 256:(t + 1) * 256], ps[:],
                            AF.Identity, bias=bq_sb[:, ct:ct + 1])

            # ============ Phase B: attention + projection ============
            phase_b = cfg["cut"] != 1
            with (
                tc.tile_pool(name="suf", bufs=1) as sufp,
                tc.tile_pool(name="mstr", bufs=3) as mstr,
                tc.tile_pool(name="eb", bufs=2) as ebp,
                tc.tile_pool(name="zr", bufs=2) as zrp,
                tc.tile_pool(name="at", bufs=2) as atp,
                tc.tile_pool(name="wop", bufs=2) as wop,
                tc.tile_pool(name="outp", bufs=2) as outp,
                tc.tile_pool(name="pss", bufs=2, space="PSUM") as pss,
                tc.tile_pool(name="psav", bufs=1, space="PSUM") as psav,
                tc.tile_pool(name="pspj", bufs=1, space="PSUM") as pspj,
            ):
                if use_ag and phase_b:
                    for ct in range(8):
                        for r in range(4):
                            nc.sync.dma_start(
                                kt[:, ct, 512 * r:512 * r + 512],
                                kout_dr[1024 * r + 128 * ct:
                                        1024 * r + 128 * (ct + 1), :])
                    for c in range(NKC):
                        nc.sync.dma_start(
                            v[:, c, :], vout_dr[c * 128:(c + 1) * 128, :])

                # suffix vector for tile A: 1/16 * colsum_{c>=8} v_c
                suf = sufp.tile([1, E], BF)
                for half in range(2) if phase_b else ():
                    psS = pspj.tile([1, 512], FP, tag="psS")
                    for c in range(NA, NKC):
                        nc.tensor.matmul(
                            psS[:], o16_sb[:, 0:1],
                            v[:, c, half * 512:(half + 1) * 512],
                            start=(c == NA), stop=(c == NKC - 1))
                    nc.scalar.activation(
                        suf[:, half * 512:(half + 1) * 512], psS[:], AF.Copy)

                if cfg["cut"] in (1, 2):
                    tiles_iter = ()
                else:
                    tiles_iter = ((0, NA), (1, NB))
                for t, nchunks in tiles_iter:
                    av = [psav.tile([128, 512], FP, tag=f"av{gb}",
                                    name=f"av{gb}") for gb in range(4)]
                    for gb in range(4):
                        nc.tensor.matmul(av[gb][:], ones_sb[:1, :128],
                                         zeros_sb[:1, :], start=True,
                                         stop=False, skip_group_check=True)
                    for c in range(nchunks):
                        slot = t * NA + c if t == 0 else NA + c
                        mm_t = mstr.tile([128, QT], BF, tag="mm")
                        mz_t = mstr.tile([128, QT], BF, tag="mzt")
                        m1_t = mstr.tile([128, QT], BF, tag="m1t")
                        nc.sync.dma_start(mm_t[:], m_d[slot, :, :])
                        nc.sync.dma_start(mz_t[:], mz_d[slot, :, :])
                        nc.sync.dma_start(m1_t[:], m1s_d[slot, :, :])
                        eb = ebp.tile([128, H, QT], BF, tag="eb")
                        for h in range(H):
                            po = (h % 2) * 64
                            ps = pss.tile([128, QT], FP, tag="ps")
                            nc.tensor.matmul(
                                ps[:],
                                kt[po:po + 64, h // 2, c * KC:(c + 1) * KC],
                                qt[po:po + 64, h // 2, t * QT:(t + 1) * QT],
                                start=True, stop=True)
                            nc.scalar.activation(eb[:, h, :], ps[:], AF.Exp)
                        # T = sum over heads; D = T + mz; rbm = m/D
                        z8 = zrp.tile([128, 8, QT], BF, tag="z8")
                        nc.vector.tensor_add(z8[:], eb[:, 0:8, :],
                                             eb[:, 8:16, :])
                        z4 = zrp.tile([128, 4, QT], BF, tag="z4")
                        nc.vector.tensor_add(z4[:], z8[:, 0:4, :],
                                             z8[:, 4:8, :])
                        z2 = zrp.tile([128, 2, QT], BF, tag="z2")
                        nc.vector.tensor_add(z2[:], z4[:, 0:2, :],
                                             z4[:, 2:4, :])
                        zf = zrp.tile([128, QT], FP, tag="zf")
                        nc.vector.tensor_add(zf[:], z2[:, 0, :], z2[:, 1, :])
                        d = zrp.tile([128, QT], FP, tag="d")
                        nc.vector.tensor_add(d[:], zf[:], mz_t[:])
                        r = zrp.tile([128, QT], FP, tag="r")
                        nc.vector.reciprocal(r[:], d[:])
                        rbm = zrp.tile([128, QT], BF, tag="rbm")
                        nc.vector.tensor_mul(rbm[:], r[:], mm_t[:])
                        if cfg["bcast_norm"]:
                            nc.vector.tensor_mul(eb[:], eb[:],
                                                 _bcast_mid(rbm[:], H))
                        else:
                            for h in range(H):
                                nc.vector.tensor_mul(eb[:, h, :],
                                                     eb[:, h, :], rbm[:])
                        for h in range(H):
                            gb, g2, po = h // 4, (h // 2) % 2, (h % 2) * 64
                            nc.tensor.matmul(
                                av[gb][po:po + 64, g2 * QT:(g2 + 1) * QT],
                                v[:, c, h * 64:(h + 1) * 64], eb[:, h, :],
                                start=False, stop=False,
                                skip_group_check=True)
                            if not cfg["no_corr"]:
                                nc.tensor.matmul(
                                    av[gb][po:po + 64, g2 * QT:(g2 + 1) * QT],
                                    v[:, c, h * 64:(h + 1) * 64], m1_t[:],
                                    start=False, stop=False,
                                    skip_group_check=True)
                    if t == 0 and not cfg["no_suffix"]:
                        for gb in range(4):
                            for g2 in range(2):
                                nc.tensor.matmul(
                                    av[gb][:, g2 * QT:(g2 + 1) * QT],
                                    suf[:1, 256 * gb + 128 * g2:
                                        256 * gb + 128 * (g2 + 1)],
                                    ones_sb[:1, :], start=False, stop=False,
                                    skip_group_check=True)
                    for gb in range(4):
                        nc.tensor.matmul(av[gb][:], ones_sb[:1, :128],
                                         zeros_sb[:1, :], start=False,
                                         stop=True, skip_group_check=True)
                    # aT + projection for this tile
                    at = atp.tile([128, 8, QT], BF, tag="at")
                    for e in range(8):
                        nc.scalar.activation(
                            at[:, e, :],
                            av[e // 2][:, (e % 2) * QT:(e % 2 + 1) * QT],
                            AF.Copy)
                    for cc in range(2):
                        wocc = wop.tile([128, 8, 512], BF, tag="wocc")
                        nc.sync.dma_start(
                            wocc[:],
                            wo_d[:, cc * 512:(cc + 1) * 512].rearrange(
                                "(e p) c -> p e c", p=128))
                        for qs in range(2):
                            ps = pspj.tile([128, 512], FP, tag="pj")
                            for e in range(8):
                                nc.tensor.matmul(
                                    ps[:],
                                    at[:, e, qs * 128:(qs + 1) * 128],
                                    wocc[:, e, :],
                                    start=(e == 0), stop=False)
                            nc.tensor.matmul(
                                ps[:], ones_sb[:1, :128],
                                bo_sb[:1, cc * 512:(cc + 1) * 512],
                                start=False, stop=True)
                            ot = outp.tile([128, 512], FP, tag="ot")
                            nc.scalar.activation(ot[:], ps[:], AF.Copy)
                            nc.sync.dma_start(
                                out_d[t * QT + qs * 128:
                                      t * QT + (qs + 1) * 128,
                                      cc * 512:(cc + 1) * 512], ot[:])

            if loop is not None:
                loop.__exit__(None, None, None)

    nc.compile()
    return nc


def prep_inputs(x, W_atten, b_atten, W_proj, b_proj, cfg=None):
    cfg = dict(CFG, **(cfg or {}))
    x = np.asarray(x, dtype=np.float32)
    W3 = np.asarray(W_atten, dtype=np.float32).reshape(E, H, 3, HD)
    b3 = np.asarray(b_atten, dtype=np.float32).reshape(H, 3, HD)
    wq = np.ascontiguousarray(W3[:, :, 0, :].reshape(E, E)).astype(NPBF)
    wk = np.ascontiguousarray(W3[:, :, 1, :].reshape(E, E)).astype(NPBF)
    wv = np.ascontiguousarray(W3[:, :, 2, :].reshape(E, E)).astype(NPBF)
    bq = np.ascontiguousarray(b3[:, 0, :].reshape(E).reshape(8, 128).T)
    bk = np.ascontiguousarray(b3[:, 1, :].reshape(E).reshape(8, 128).T)
    bv = b3[:, 2, :].reshape(1, E).astype(NPBF)
    wo = np.asarray(W_proj, dtype=np.float32).astype(NPBF)
    bo = np.asarray(b_proj, dtype=np.float32).reshape(1, E).astype(NPBF)

    in_maps = []
    for core in range(N_CORES):
        b, g = core // 4, core % 4
        xb = x[b]
        xTq = np.concatenate(
            [xb[256 * g:256 * g + 256, :].T,
             xb[256 * (g + 4):256 * (g + 4) + 256, :].T],
            axis=1)
        xTk = np.ascontiguousarray(xb[512 * g:512 * g + 512, :].T)
        # masks for 24 slots: slot s -> (tile t, chunk c)
        m = np.zeros((24, KC, QT), np.float32)
        for s in range(24):
            t, c = (0, s) if s < NA else (1, s - NA)
            j = g if t == 0 else g + 4
            qg = QT * j + np.arange(QT)[None, :]
            kg = KC * c + np.arange(KC)[:, None]
            m[s] = (qg >= kg).astype(np.float32)
        im = {
            "xTq": np.ascontiguousarray(xTq).astype(NPBF),
            "xTk": xTk.astype(NPBF),
            "wq": wq, "wk": wk, "wv": wv, "wo": wo,
            "bq": bq, "bk": bk, "bv": bv, "bo": bo,
            "onesr": np.ones((1, 256), NPBF),
            "zerosr": np.zeros((1, 512), NPBF),
            "o16": np.full((128, 1), 1.0 / 16, NPBF),
            "mm": m.astype(NPBF),
            "mz": (16.0 * (1.0 - m)).astype(NPBF),
            "m1s": ((1.0 - m) / 16.0).astype(NPBF),
        }
        if not cfg["use_ag"]:
            im["xT"] = np.ascontiguousarray(xb.T).astype(NPBF)
        in_maps.append(im)
    return in_maps


def kernel(x, W_atten, b_atten, W_proj, b_proj):
    nc = build_program(reps=1)
    in_maps = prep_inputs(x, W_atten, b_atten, W_proj, b_proj)
    res = bass_utils.run_bass_kernel_spmd(
        nc, in_maps, core_ids=list(range(N_CORES)))
    out = np.empty((B, S, E), dtype=np.float32)
    for core in range(N_CORES):
        b, g = core // 4, core % 4
        out[b, 256 * g:256 * g + 256] = res.results[core]["out"][0:256]
        out[b, 256 * (g + 4):256 * (g + 4) + 256] = \
            res.results[core]["out"][256:512]
    return out


# revision 4
# speedup vs baseline: 1.0531x; 1.0531x over previous
"""Trainium2 Bass kernel v2 for nn_Attention_52982716563627.

Reference math (quirks preserved): qkv = x@W_atten + b; per-head scores
s = q k^T (no 1/sqrt(dk)); mask as w*m + (1-m)*1e5; softmax over the HEAD
axis; a = p@v; out = a@W_proj + b_proj.

Softmax-over-heads identities used:
  * masked (k,i): all 16 heads equal -> p = 1/16 exactly.
  * e = exp(s) computed raw; T = sum_h e; D = T + mz where mz = 16*(1-m)
    (masked entries: D irrelevant-but-finite); rbm = m/D; p_h = e_h*rbm
    (0 on masked entries).
  * the missing masked mass (1-m)/16 enters via a per-chunk correction
    matmul  av += v_chunk^T @ m1s  with m1s = (1-m)/16  (exact), and for
    slot-A chunks c >= 8 via a rank-1 suffix term (1/16 * colsum v).

Uniform SPMD program (same NEFF on 8 cores; per-core behavior via data):
  core c: batch b = c//4, rank g = c%4.
  q tile A = rows [256g, 256g+256)        -> 8 key chunks + suffix(c>=8)
  q tile B = rows [256(g+4), 256(g+4)+256) -> 16 key chunks
  K/V: core computes keys [512g, 512g+512), AllGather across the 4-core
  batch group (overlaps q projection).  All matmul operands bf16
  (PE 1 cyc/row + FWL); accumulation fp32 in PSUM; e/p bf16 (2x DVE).
"""

import numpy as np
import ml_dtypes

import concourse.bass as bass
import concourse.bacc as bacc
import concourse.mybir as mybir
import concourse.tile as tile
from concourse import bass_utils

N_CORES = 8
B, S, E = 2, 2048, 1024
H, HD = 16, 64
QT = 256
KC = 128
NKC = 16
NA, NB = 8, 16          # chunks processed for tile A / tile B
FP = mybir.dt.float32
BF = mybir.dt.bfloat16
AF = mybir.ActivationFunctionType
NPBF = ml_dtypes.bfloat16

CFG = dict(
    use_ag=False,
    bcast_norm=True,
    no_corr=False,   # debug: skip m1 correction matmuls
    no_suffix=False, # debug: skip suffix rank-1 adds
    cut=0,  # 1: phase A only; 2: +kt/v load+suffix only
)


def _bcast_mid(ap, n):
    """View a [128, Q] AP as [128, n, Q] with 0-stride middle dim."""
    return bass.AP(tensor=ap.tensor, offset=ap.offset,
                   ap=[ap.ap[0], [0, n], ap.ap[1]])


def build_program(reps: int = 1, cfg=None):
    cfg = dict(CFG, **(cfg or {}))
    use_ag = cfg["use_ag"]
    nc = bacc.Bacc("TRN2", target_bir_lowering=False, debug=False,
                   num_devices=N_CORES)

    xTq_d = nc.dram_tensor("xTq", [E, 512], BF, kind="ExternalInput")
    xTk_d = nc.dram_tensor("xTk", [E, 512], BF, kind="ExternalInput")
    if not use_ag:
        xT_d = nc.dram_tensor("xT", [E, S], BF, kind="ExternalInput")
    wq_d = nc.dram_tensor("wq", [E, E], BF, kind="ExternalInput")
    wk_d = nc.dram_tensor("wk", [E, E], BF, kind="ExternalInput")
    wv_d = nc.dram_tensor("wv", [E, E], BF, kind="ExternalInput")
    wo_d = nc.dram_tensor("wo", [E, E], BF, kind="ExternalInput")
    bq_d = nc.dram_tensor("bq", [128, 8], FP, kind="ExternalInput")
    bk_d = nc.dram_tensor("bk", [128, 8], FP, kind="ExternalInput")
    bv_d = nc.dram_tensor("bv", [1, E], BF, kind="ExternalInput")
    bo_d = nc.dram_tensor("bo", [1, E], BF, kind="ExternalInput")
    ones_d = nc.dram_tensor("onesr", [1, 256], BF, kind="ExternalInput")
    zeros_d = nc.dram_tensor("zerosr", [1, 512], BF, kind="ExternalInput")
    o16_d = nc.dram_tensor("o16", [128, 1], BF, kind="ExternalInput")
    m_d = nc.dram_tensor("mm", [24, KC, QT], BF, kind="ExternalInput")
    mz_d = nc.dram_tensor("mz", [24, KC, QT], BF, kind="ExternalInput")
    m1s_d = nc.dram_tensor("m1s", [24, KC, QT], BF, kind="ExternalInput")
    out_d = nc.dram_tensor("out", [512, E], FP, kind="ExternalOutput")

    with tile.TileContext(nc) as tc:
        with (
            tc.tile_pool(name="consts", bufs=1) as consts,
            tc.tile_pool(name="qt", bufs=1) as qtp,
            tc.tile_pool(name="vsb", bufs=1) as vsbp,
            tc.tile_pool(name="ktp", bufs=1) as ktpool,
            tc.tile_pool(name="dram", bufs=1, space="DRAM") as dramp,
        ):
            ones_sb = consts.tile([1, 256], BF)
            zeros_sb = consts.tile([1, 512], BF)
            o16_sb = consts.tile([128, 1], BF)
            bq_sb = consts.tile([128, 8], FP)
            bk_sb = consts.tile([128, 8], FP)
            bv_sb = consts.tile([1, E], BF)
            bo_sb = consts.tile([1, E], BF)
            nc.sync.dma_start(ones_sb[:], ones_d[:])
            nc.sync.dma_start(zeros_sb[:], zeros_d[:])
            nc.sync.dma_start(o16_sb[:], o16_d[:])
            nc.sync.dma_start(bq_sb[:], bq_d[:])
            nc.sync.dma_start(bk_sb[:], bk_d[:])
            nc.sync.dma_start(bv_sb[:], bv_d[:])
            nc.sync.dma_start(bo_sb[:], bo_d[:])

            qt = qtp.tile([128, 8, 512], BF)
            v = vsbp.tile([128, NKC, E], BF)
            kt = ktpool.tile([128, 8, S], BF)

            if use_ag:
                kin_dr = dramp.tile([E, 512], BF)
                kout_dr = dramp.tile([4 * E, 512], BF)
                vin_dr = dramp.tile([512, E], BF)
                vout_dr = dramp.tile([S, E], BF)

            loop = tc.For_i(0, reps, 1) if reps > 1 else None
            if loop is not None:
                loop.__enter__()

            # ============ Phase A: own K/V slice -> AG; q proj ============
            with (
                tc.tile_pool(name="xts", bufs=1) as xtsp,
                tc.tile_pool(name="wstr", bufs=2) as wstr,
                tc.tile_pool(name="kvo", bufs=3) as kvop,
                tc.tile_pool(name="pskq", bufs=2, space="PSUM") as pskq,
                tc.tile_pool(name="psv", bufs=2, space="PSUM") as psv,
            ):
                if use_ag:
                    xtk = xtsp.tile([128, 8, 512], BF, name="xtk")
                    for e in range(8):
                        nc.sync.dma_start(
                            xtk[:, e, :], xTk_d[e * 128:(e + 1) * 128, :])
                else:
                    xt = xtsp.tile([128, 8, S], BF, name="xt")
                    for e in range(8):
                        nc.sync.dma_start(
                            xt[:, e, :], xT_d[e * 128:(e + 1) * 128, :])
                xtq = xtsp.tile([128, 8, 512], BF, name="xtq")
                for e in range(8):
                    nc.sync.dma_start(
                        xtq[:, e, :], xTq_d[e * 128:(e + 1) * 128, :])

                # own kT slice (AG) or full kT (fallback)
                for ct in range(8):
                    wct = wstr.tile([128, 8, 128], BF, tag="wct")
                    nc.sync.dma_start(
                        wct[:],
                        wk_d[:, ct * 128:(ct + 1) * 128].rearrange(
                            "(e p) c -> p e c", p=128))
                    if use_ag:
                        ps = pskq.tile([128, 512], FP, tag="pkq")
                        for e in range(8):
                            nc.tensor.matmul(ps[:], wct[:, e, :],
                                             xtk[:, e, :],
                                             start=(e == 0), stop=(e == 7))
                        ktp = kvop.tile([128, 512], BF, tag="ktp")
                        nc.scalar.activation(ktp[:], ps[:], AF.Identity,
                                             bias=bk_sb[:, ct:ct + 1])
                        nc.sync.dma_start(
                            kin_dr[ct * 128:(ct + 1) * 128, :], ktp[:])
                    else:
                        for kc4 in range(4):
                            ps = pskq.tile([128, 512], FP, tag="pkq")
                            for e in range(8):
                                nc.tensor.matmul(
                                    ps[:], wct[:, e, :],
                                    xt[:, e, kc4 * 512:(kc4 + 1) * 512],
                                    start=(e == 0), stop=(e == 7))
                            nc.scalar.activation(
                                kt[:, ct, kc4 * 512:(kc4 + 1) * 512], ps[:],
                                AF.Identity, bias=bk_sb[:, ct:ct + 1])
                if use_ag:
                    nc.gpsimd.collective_compute(
                        "AllGather", mybir.AluOpType.bypass,
                        replica_groups=[[0, 1, 2, 3], [4, 5, 6, 7]],
                        ins=[kin_dr.opt()], outs=[kout_dr.opt()])

                # own v slice (AG) or full v (fallback)
                for cc in range(2):
                    wvcc = wstr.tile([128, 8, 512], BF, tag="wvcc")
                    nc.sync.dma_start(
                        wvcc[:],
                        wv_d[:, cc * 512:(cc + 1) * 512].rearrange(
                            "(e p) c -> p e c", p=128))
                    nrt = 4 if use_ag else 16
                    for rt in range(nrt):
                        ps = psv.tile([128, 512], FP, tag="pv")
                        for e in range(8):
                            src = (xtk[:, e, rt * 128:(rt + 1) * 128]
                                   if use_ag else
                                   xt[:, e, rt * 128:(rt + 1) * 128])
                            nc.tensor.matmul(ps[:], src, wvcc[:, e, :],
                                             start=(e == 0), stop=False)
                        nc.tensor.matmul(ps[:], ones_sb[:1, :128],
                                         bv_sb[:1, cc * 512:(cc + 1) * 512],
                                         start=False, stop=True)
                        vo = kvop.tile([128, 512], BF, tag="vo")
                        nc.scalar.activation(vo[:], ps[:], AF.Copy)
                        if use_ag:
                            nc.sync.dma_start(
                                vin_dr[rt * 128:(rt + 1) * 128,
                                       cc * 512:(cc + 1) * 512], vo[:])
                        else:
                            nc.vector.tensor_copy(
                                v[:, rt, cc * 512:(cc + 1) * 512], vo[:])
                if use_ag:
                    nc.gpsimd.collective_compute(
                        "AllGather", mybir.AluOpType.bypass,
                        replica_groups=[[0, 1, 2, 3], [4, 5, 6, 7]],
                        ins=[vin_dr.opt()], outs=[vout_dr.opt()])

                # q projection for the two own 256-row tiles
                for ct in range(8):
                    wct = wstr.tile([128, 8, 128], BF, tag="wct")
                    nc.sync.dma_start(
                        wct[:],
                        wq_d[:, ct * 128:(ct + 1) * 128].rearrange(
                            "(e p) c -> p e c", p=128))
                    for t in range(2):
                        ps = pskq.tile([128, 256], FP, tag="pq")
                        for e in range(8):
                            nc.tensor.matmul(
                                ps[:], wct[:, e, :],
                                xtq[:, e, t * 256:(t + 1) * 256],
                                start=(e == 0), stop=(e == 7))
                        nc.scalar.activation(
                            qt[:, ct, t * 256:(t + 1) * 256], ps[:],
                            AF.Identity, bias=bq_sb[:, ct:ct + 1])

            # ============ Phase B: attention + projection ============
            phase_b = cfg["cut"] != 1
            with (
                tc.tile_pool(name="suf", bufs=1) as sufp,
                tc.tile_pool(name="mstr", bufs=3) as mstr,
                tc.tile_pool(name="eb", bufs=2) as ebp,
                tc.tile_pool(name="zr", bufs=2) as zrp,
                tc.tile_pool(name="at", bufs=2) as atp,
                tc.tile_pool(name="wop", bufs=2) as wop,
                tc.tile_pool(name="outp", bufs=2) as outp,
                tc.tile_pool(name="pss", bufs=2, space="PSUM") as pss,
                tc.tile_pool(name="psav", bufs=1, space="PSUM") as psav,
                tc.tile_pool(name="pspj", bufs=1, space="PSUM") as pspj,
            ):
                if use_ag and phase_b:
                    for ct in range(8):
                        for r in range(4):
                            nc.sync.dma_start(
                                kt[:, ct, 512 * r:512 * r + 512],
                                kout_dr[1024 * r + 128 * ct:
                                        1024 * r + 128 * (ct + 1), :])
                    for c in range(NKC):
                        nc.sync.dma_start(
                            v[:, c, :], vout_dr[c * 128:(c + 1) * 128, :])

                # suffix vector for tile A: 1/16 * colsum_{c>=8} v_c
                suf = sufp.tile([1, E], BF)
                for half in range(2) if phase_b else ():
                    psS = pspj.tile([1, 512], FP, tag="psS")
                    for c in range(NA, NKC):
                        nc.tensor.matmul(
                            psS[:], o16_sb[:, 0:1],
                            v[:, c, half * 512:(half + 1) * 512],
                            start=(c == NA), stop=(c == NKC - 1))
                    nc.scalar.activation(
                        suf[:, half * 512:(half + 1) * 512], psS[:], AF.Copy)

                if cfg["cut"] in (1, 2):
                    tiles_iter = ()
                else:
                    tiles_iter = ((0, NA), (1, NB))
                for t, nchunks in tiles_iter:
                    av = [psav.tile([128, 512], FP, tag=f"av{gb}",
                                    name=f"av{gb}") for gb in range(4)]
                    for gb in range(4):
                        nc.tensor.matmul(av[gb][:], ones_sb[:1, :128],
                                         zeros_sb[:1, :], start=True,
                                         stop=False, skip_group_check=True)
                    for c in range(nchunks):
                        slot = t * NA + c if t == 0 else NA + c
                        mm_t = mstr.tile([128, QT], BF, tag="mm")
                        mz_t = mstr.tile([128, QT], BF, tag="mzt")
                        m1_t = mstr.tile([128, QT], BF, tag="m1t")
                        nc.sync.dma_start(mm_t[:], m_d[slot, :, :])
                        nc.sync.dma_start(mz_t[:], mz_d[slot, :, :])
                        nc.sync.dma_start(m1_t[:], m1s_d[slot, :, :])
                        eb = ebp.tile([128, H, QT], BF, tag="eb")
                        for h in range(H):
                            po = (h % 2) * 64
                            ps = pss.tile([128, QT], FP, tag="ps")
                            nc.tensor.matmul(
                                ps[:],
                                kt[po:po + 64, h // 2, c * KC:(c + 1) * KC],
                                qt[po:po + 64, h // 2, t * QT:(t + 1) * QT],
                                start=True, stop=True)
                            nc.scalar.activation(eb[:, h, :], ps[:], AF.Exp)
                        # T = sum over heads; D = T + mz; rbm = m/D
                        z8 = zrp.tile([128, 8, QT], BF, tag="z8")
                        nc.vector.tensor_add(z8[:], eb[:, 0:8, :],
                                             eb[:, 8:16, :])
                        z4 = zrp.tile([128, 4, QT], BF, tag="z4")
                        nc.vector.tensor_add(z4[:], z8[:, 0:4, :],
                                             z8[:, 4:8, :])
                        z2 = zrp.tile([128, 2, QT], BF, tag="z2")
                        nc.vector.tensor_add(z2[:], z4[:, 0:2, :],
                                             z4[:, 2:4, :])
                        zf = zrp.tile([128, QT], FP, tag="zf")
                        nc.vector.tensor_add(zf[:], z2[:, 0, :], z2[:, 1, :])
                        d = zrp.tile([128, QT], FP, tag="d")
                        nc.vector.tensor_add(d[:], zf[:], mz_t[:])
                        r = zrp.tile([128, QT], FP, tag="r")
                        nc.vector.reciprocal(r[:], d[:])
                        rbm = zrp.tile([128, QT], BF, tag="rbm")
                        nc.vector.tensor_mul(rbm[:], r[:], mm_t[:])
                        if cfg["bcast_norm"]:
                            nc.vector.tensor_mul(eb[:], eb[:],
                                                 _bcast_mid(rbm[:], H))
                        else:
                            for h in range(H):
                                nc.vector.tensor_mul(eb[:, h, :],
                                                     eb[:, h, :], rbm[:])
                        for h in range(H):
                            gb, g2, po = h // 4, (h // 2) % 2, (h % 2) * 64
                            nc.tensor.matmul(
                                av[gb][po:po + 64, g2 * QT:(g2 + 1) * QT],
                                v[:, c, h * 64:(h + 1) * 64], eb[:, h, :],
                                start=False, stop=False,
                                skip_group_check=True)
                            if not cfg["no_corr"]:
                                nc.tensor.matmul(
                                    av[gb][po:po + 64, g2 * QT:(g2 + 1) * QT],
                                    v[:, c, h * 64:(h + 1) * 64], m1_t[:],
                                    start=False, stop=False,
                                    skip_group_check=True)
                    if t == 0 and not cfg["no_suffix"]:
                        for gb in range(4):
                            for g2 in range(2):
                                nc.tensor.matmul(
                                    av[gb][:, g2 * QT:(g2 + 1) * QT],
                                    suf[:1, 256 * gb + 128 * g2:
                                        256 * gb + 128 * (g2 + 1)],
                                    ones_sb[:1, :], start=False, stop=False,
                                    skip_group_check=True)
                    for gb in range(4):
                        nc.tensor.matmul(av[gb][:], ones_sb[:1, :128],
                                         zeros_sb[:1, :], start=False,
                                         stop=True, skip_group_check=True)
                    # aT + projection for this tile
                    at = atp.tile([128, 8, QT], BF, tag="at")
                    for e in range(8):
                        nc.scalar.activation(
                            at[:, e, :],
                            av[e // 2][:, (e % 2) * QT:(e % 2 + 1) * QT],
                            AF.Copy)
                    for cc in range(2):
                        wocc = wop.tile([128, 8, 512], BF, tag="wocc")
                        nc.sync.dma_start(
                            wocc[:],
                            wo_d[:, cc * 512:(cc + 1) * 512].rearrange(
                                "(e p) c -> p e c", p=128))
                        for qs in range(2):
                            ps = pspj.tile([128, 512], FP, tag="pj")
                            for e in range(8):
                                nc.tensor.matmul(
                                    ps[:],
                                    at[:, e, qs * 128:(qs + 1) * 128],
                                    wocc[:, e, :],
                                    start=(e == 0), stop=False)
                            nc.tensor.matmul(
                                ps[:], ones_sb[:1, :128],
                                bo_sb[:1, cc * 512:(cc + 1) * 512],
                                start=False, stop=True)
                            ot = outp.tile([128, 512], FP, tag="ot")
                            nc.scalar.activation(ot[:], ps[:], AF.Copy)
                            nc.sync.dma_start(
                                out_d[t * QT + qs * 128:
                                      t * QT + (qs + 1) * 128,
                                      cc * 512:(cc + 1) * 512], ot[:])

            if loop is not None:
                loop.__exit__(None, None, None)

    nc.compile()
    return nc


def prep_inputs(x, W_atten, b_atten, W_proj, b_proj, cfg=None):
    cfg = dict(CFG, **(cfg or {}))
    x = np.asarray(x, dtype=np.float32)
    W3 = np.asarray(W_atten, dtype=np.float32).reshape(E, H, 3, HD)
    b3 = np.asarray(b_atten, dtype=np.float32).reshape(H, 3, HD)
    wq = np.ascontiguousarray(W3[:, :, 0, :].reshape(E, E)).astype(NPBF)
    wk = np.ascontiguousarray(W3[:, :, 1, :].reshape(E, E)).astype(NPBF)
    wv = np.ascontiguousarray(W3[:, :, 2, :].reshape(E, E)).astype(NPBF)
    bq = np.ascontiguousarray(b3[:, 0, :].reshape(E).reshape(8, 128).T)
    bk = np.ascontiguousarray(b3[:, 1, :].reshape(E).reshape(8, 128).T)
    bv = b3[:, 2, :].reshape(1, E).astype(NPBF)
    wo = np.asarray(W_proj, dtype=np.float32).astype(NPBF)
    bo = np.asarray(b_proj, dtype=np.float32).reshape(1, E).astype(NPBF)

    in_maps = []
    for core in range(N_CORES):
        b, g = core // 4, core % 4
        xb = x[b]
        xTq = np.concatenate(
            [xb[256 * g:256 * g + 256, :].T,
             xb[256 * (g + 4):256 * (g + 4) + 256, :].T],
            axis=1)
        xTk = np.ascontiguousarray(xb[512 * g:512 * g + 512, :].T)
        # masks for 24 slots: slot s -> (tile t, chunk c)
        m = np.zeros((24, KC, QT), np.float32)
        for s in range(24):
            t, c = (0, s) if s < NA else (1, s - NA)
            j = g if t == 0 else g + 4
            qg = QT * j + np.arange(QT)[None, :]
            kg = KC * c + np.arange(KC)[:, None]
            m[s] = (qg >= kg).astype(np.float32)
        im = {
            "xTq": np.ascontiguousarray(xTq).astype(NPBF),
            "xTk": xTk.astype(NPBF),
            "wq": wq, "wk": wk, "wv": wv, "wo": wo,
            "bq": bq, "bk": bk, "bv": bv, "bo": bo,
            "onesr": np.ones((1, 256), NPBF),
            "zerosr": np.zeros((1, 512), NPBF),
            "o16": np.full((128, 1), 1.0 / 16, NPBF),
            "mm": m.astype(NPBF),
            "mz": (16.0 * (1.0 - m)).astype(NPBF),
            "m1s": ((1.0 - m) / 16.0).astype(NPBF),
        }
        if not cfg["use_ag"]:
            im["xT"] = np.ascontiguousarray(xb.T).astype(NPBF)
        in_maps.append(im)
    return in_maps


def kernel(x, W_atten, b_atten, W_proj, b_proj):
    nc = build_program(reps=1)
    in_maps = prep_inputs(x, W_atten, b_atten, W_proj, b_proj)
    res = bass_utils.run_bass_kernel_spmd(
        nc, in_maps, core_ids=list(range(N_CORES)))
    out = np.empty((B, S, E), dtype=np.float32)
    for core in range(N_CORES):
        b, g = core // 4, core % 4
        out[b, 256 * g:256 * g + 256] = res.results[core]["out"][0:256]
        out[b, 256 * (g + 4):256 * (g + 4) + 256] = \
            res.results[core]["out"][256:512]
    return out
